# revision 30
# baseline (speedup 1.0000x reference)
"""Balanced grouped-expert SwiGLU kernel v2.

Tokens tile-balanced across cores at MT=256 granularity (17 slots/core
for the reference counts, 6.3% padding vs 12.5% at MT=512). A core's
slots span at most 2 experts (A then B); the switch index Ta is runtime
data -> per-slot tc.If/Else picks resident weight set. Only the PE
matmuls live inside branches; DMAs, silu, muls and copies are hoisted
out (identical either way).

Schedule: software-pipelined X (x1/x3 GEMMs + SwiGLU) one slot ahead of
G (down-projection GEMM), so the PE never waits on the DVE chain that
produces ht. DMA: meta first, then weight f-chunks and x tiles in
need-order on the SP queue; output stores go on the Activation queue.
Host pre-transposes all DRAM layouts so every DMA is 128 contiguous
per-partition runs. PSUM: 4 banks of paired x1/x3 tiles + 4 banks for
the down-proj, exactly 8.
"""

import math
import os

import ml_dtypes
import numpy as np

D = 2048
F = 512
MT = 256
KC = D // 128   # 16 k-blocks of the contraction dim
FC = F // 128   # 4 f-blocks of the ffn dim
NCORES = 8

_cache = {}


def _build(nt: int, passes: int = 1):
    import concourse.bacc as bacc
    import concourse.mybir as mybir
    from concourse.tile import TileContext

    dt = mybir.dt
    f32 = dt.float32
    bf16 = dt.bfloat16
    i32 = dt.int32
    PAD_T = nt * MT
    SIGMOID = mybir.ActivationFunctionType.Sigmoid

    nc = bacc.Bacc(
        "TRN2", target_bir_lowering=False, debug=False,
        enable_asserts=False, num_devices=NCORES,
    )

    xq = nc.dram_tensor("xq", [128, nt * KC * MT], bf16, kind="ExternalInput")
    wdr = {}
    for pre in ("a", "b"):
        wdr[pre] = (
            nc.dram_tensor(f"w{pre}1", [128, FC * KC * 128], bf16,
                           kind="ExternalInput"),
            nc.dram_tensor(f"w{pre}2", [128, FC, D], bf16,
                           kind="ExternalInput"),
            nc.dram_tensor(f"w{pre}3", [128, FC * KC * 128], bf16,
                           kind="ExternalInput"),
        )
    meta = nc.dram_tensor("meta", [1, 1], i32, kind="ExternalInput")
    out = nc.dram_tensor("out", [PAD_T, D], bf16, kind="ExternalOutput")

    with TileContext(nc) as tc:
        with (
            tc.tile_pool(name="wpool", bufs=1) as wpool,
            tc.tile_pool(name="xt", bufs=4) as xt_pool,
            tc.tile_pool(name="sil", bufs=8) as sil_pool,
            tc.tile_pool(name="ht", bufs=3) as ht_pool,
            tc.tile_pool(name="osb", bufs=4) as osb_pool,
            tc.tile_pool(name="psx", bufs=4, space="PSUM") as psx_pool,
            tc.tile_pool(name="pso", bufs=4, space="PSUM") as pso_pool,
        ):
            # meta first so the If condition register resolves immediately
            msb = wpool.tile([1, 1], i32, tag="meta")
            nc.sync.dma_start(out=msb[:], in_=meta.ap())
            ta_v = nc.snap(nc.values_load(msb[0:1, 0:1]))

            wsb = {}
            for pre in ("a", "b"):
                s1 = wpool.tile([128, FC, KC, 128], bf16, tag=f"w1{pre}")
                s3 = wpool.tile([128, FC, KC, 128], bf16, tag=f"w3{pre}")
                s2 = wpool.tile([128, FC, D], bf16, tag=f"w2{pre}")
                wsb[pre] = (s1, s2, s3)

            CH = KC * 128  # elements per f-chunk per partition

            def load_w13_chunk(pre, fc):
                s1, _, s3 = wsb[pre]
                d1, _, d3 = wdr[pre]
                if pre == "a" and fc == 0:
                    # first chunk gates kernel start: interleave w1/w3
                    # k-halves in consumption order so the k=0 Ldweights
                    # of x1 (then x3) waits only a half transfer
                    h = CH // 2
                    kk = KC // 2
                    nc.sync.dma_start(out=s1[:, 0, :kk], in_=d1[:, 0:h])
                    nc.sync.dma_start(out=s3[:, 0, :kk], in_=d3[:, 0:h])
                    nc.sync.dma_start(out=s1[:, 0, kk:], in_=d1[:, h:CH])
                    nc.sync.dma_start(out=s3[:, 0, kk:], in_=d3[:, h:CH])
                    return
                nc.sync.dma_start(
                    out=s1[:, fc], in_=d1[:, fc * CH:(fc + 1) * CH])
                nc.sync.dma_start(
                    out=s3[:, fc], in_=d3[:, fc * CH:(fc + 1) * CH])

            def load_w2(pre):
                # dc-column chunks: the first G group only needs chunk 0
                for dc in range(4):
                    nc.sync.dma_start(
                        out=wsb[pre][1][:, :, dc * 512:(dc + 1) * 512],
                        in_=wdr[pre][1][:, :, dc * 512:(dc + 1) * 512])

            xts = {}

            def load_x(m):
                xt = xt_pool.tile([128, KC, MT], bf16, tag="xt")
                base = m * KC * MT
                if m == 0:
                    # slot 0 gates kernel start: split across two queues,
                    # low-k chunks first so the k=0 matmul starts earliest
                    q = KC // 4 * MT
                    for g, eng in enumerate((nc.scalar, nc.gpsimd,
                                             nc.scalar, nc.gpsimd)):
                        eng.dma_start(
                            out=xt[:, g * (KC // 4):(g + 1) * (KC // 4), :],
                            in_=xq[:, base + g * q:base + (g + 1) * q])
                else:
                    nc.gpsimd.dma_start(
                        out=xt[:], in_=xq[:, base:base + KC * MT])
                xts[m] = xt

            # weights stream on the SP queue in need-order; x tiles go on
            # the gpsimd queue so the two run in parallel
            for fc in range(FC):
                load_w13_chunk("a", fc)
            load_x(0)
            load_x(1)
            load_w2("a")
            for fc in range(FC):
                load_w13_chunk("b", fc)
            load_x(2)
            load_x(3)
            load_w2("b")

            def x_body(m, pre, xt, px, sigs, sils, ht):
                s1, _, s3 = wsb[pre]
                for fc in range(FC):
                    x1t = px[fc][:, 0, :]
                    x3t = px[fc][:, 1, :]
                    for k in range(KC):
                        nc.tensor.matmul(x1t, s1[:, fc, k, :], xt[:, k, :],
                                         start=(k == 0), stop=(k == KC - 1))
                    for k in range(KC):
                        nc.tensor.matmul(x3t, s3[:, fc, k, :], xt[:, k, :],
                                         start=(k == 0), stop=(k == KC - 1))
                    nc.scalar.activation(sigs[fc][:], x1t, SIGMOID)
                    nc.vector.tensor_mul(sils[fc][:], x1t, sigs[fc][:])
                    nc.vector.tensor_mul(ht[:, fc, :], sils[fc][:], x3t)

            def emit_X(m):
                if m not in xts:
                    load_x(m)
                xt = xts.pop(m)
                px = [psx_pool.tile([128, 2, MT], f32, tag="px", name=f"px{fc}")
                      for fc in range(FC)]
                sigs = [sil_pool.tile([128, MT], f32, tag="sig",
                                      name=f"sig{fc}") for fc in range(FC)]
                sils = [sil_pool.tile([128, MT], f32, tag="sil",
                                      name=f"sil{fc}") for fc in range(FC)]
                ht = ht_pool.tile([128, FC, MT], bf16, tag="ht")
                with tc.If(ta_v > m) as cmp:
                    x_body(m, "a", xt, px, sigs, sils, ht)
                with cmp.Else():
                    x_body(m, "b", xt, px, sigs, sils, ht)
                return ht

            def g_body(ht, pre, pos, osb, ts):
                s2 = wsb[pre][1]
                for dc in range(4):
                    po = pos[dc]
                    for fc in range(FC):
                        nc.tensor.matmul(
                            po[:], ht[:, fc, ts * 128:(ts + 1) * 128],
                            s2[:, fc, dc * 512:(dc + 1) * 512],
                            start=(fc == 0), stop=(fc == FC - 1))
                    nc.vector.tensor_copy(osb[:, dc * 512:(dc + 1) * 512],
                                          po[:])

            def emit_G(m, ht, last=False):
                for ts in range(MT // 128):
                    pos = [pso_pool.tile([128, 512], f32, tag="po", name=f"po{dc}")
                           for dc in range(4)]
                    osb = osb_pool.tile([128, D], bf16, tag="osb")
                    with tc.If(ta_v > m) as cmp:
                        g_body(ht, "a", pos, osb, ts)
                    with cmp.Else():
                        g_body(ht, "b", pos, osb, ts)
                    r0 = m * MT + ts * 128
                    if last and ts == MT // 128 - 1:
                        # drain the final tile in chunks so the tail is
                        # one copy + one small store, not the whole row
                        for dc in range(4):
                            nc.scalar.dma_start(
                                out=out[r0:r0 + 128, dc * 512:(dc + 1) * 512],
                                in_=osb[:, dc * 512:(dc + 1) * 512])
                    else:
                        nc.scalar.dma_start(out=out[r0:r0 + 128, :],
                                            in_=osb[:])

            for p in range(passes):
                if p > 0:
                    xts.clear()
                prev = None
                for m in range(nt):
                    ht = emit_X(m)
                    if prev is not None:
                        emit_G(m - 1, prev)
                    prev = ht
                emit_G(nt - 1, prev, last=(p == passes - 1))

    nc.compile()
    return nc


def _get_program(nt: int, passes: int = 1):
    key = (nt, passes)
    if key not in _cache:
        _cache[key] = _build(nt, passes)
    return _cache[key]


def _assign(counts):
    """Greedy: chunk the padded-tile list into per-core runs of <=NT tiles
    spanning <=2 experts. Returns (nt, per-core list of (expert, tile_lo,
    n_tiles) segment pairs) or None if infeasible."""
    E = len(counts)
    pt = [max(1, math.ceil(c / MT)) if c > 0 else 0 for c in counts]
    total = sum(pt)
    nt = math.ceil(total / NCORES)
    for nt_try in (nt, nt + 1):
        segs = [[] for _ in range(NCORES)]
        e, used = 0, 0
        for c in range(NCORES):
            cap = nt_try
            nexp = 0
            while cap > 0 and e < E:
                if pt[e] - used == 0:
                    e += 1
                    used = 0
                    continue
                if nexp == 2:
                    break
                take = min(cap, pt[e] - used)
                segs[c].append((e, used, take))
                used += take
                cap -= take
                nexp += 1
        leftover = total - sum(s[2] for core in segs for s in core)
        if leftover == 0:
            return nt_try, segs
    return None


def _prep_weights(w1, w2, w3):
    """Per-expert DRAM layouts: w1/w3 -> [128, FC*KC*128] with
    q[p, fc, k, fi] = w[k*128+p, fc*128+fi]; w2 -> [128, FC*D] with
    q[p, fc, d] = w2[fc*128+p, d]."""
    E = w1.shape[0]
    w1q, w2q, w3q = [], [], []
    for e in range(E):
        a = w1[e].astype(ml_dtypes.bfloat16).reshape(KC, 128, FC, 128)
        w1q.append(np.ascontiguousarray(
            a.transpose(1, 2, 0, 3)).reshape(128, FC * KC * 128))
        a = w3[e].astype(ml_dtypes.bfloat16).reshape(KC, 128, FC, 128)
        w3q.append(np.ascontiguousarray(
            a.transpose(1, 2, 0, 3)).reshape(128, FC * KC * 128))
        a = w2[e].astype(ml_dtypes.bfloat16).reshape(FC, 128, D)
        w2q.append(np.ascontiguousarray(
            a.transpose(1, 0, 2)).reshape(128, FC * D))
    return w1q, w2q, w3q


def kernel(x, num_tokens_per_expert, w1, w2, w3):
    from concourse.bass_utils import run_bass_kernel_spmd

    x = np.asarray(x)
    counts = [int(v) for v in np.asarray(num_tokens_per_expert)]
    w1 = np.asarray(w1)
    w2 = np.asarray(w2)
    w3 = np.asarray(w3)
    T, E = x.shape[0], len(counts)
    assert E == NCORES
    starts = np.concatenate([[0], np.cumsum(counts)])[:E].astype(np.int64)

    plan = _assign(counts)
    if plan is None:
        # fallback: expert-parallel (1 segment per core), padded to max tiles
        pt = [max(1, math.ceil(c / MT)) if c > 0 else 0 for c in counts]
        nt = max(pt)
        segs = [[(e, 0, pt[e])] if pt[e] else [] for e in range(NCORES)]
    nt, segs = (plan if plan is not None else (nt, segs))
    nc = _get_program(nt)
    PAD_T = nt * MT

    w1q, w2q, w3q = _prep_weights(w1, w2, w3)
    xT = np.ascontiguousarray(x.T).astype(ml_dtypes.bfloat16)  # [D, T]

    in_maps = []
    placements = []  # per core: list of (slot, src_lo, n_rows)
    for c in range(NCORES):
        xpt = np.zeros((D, PAD_T), dtype=ml_dtypes.bfloat16)
        place = []
        slot = 0
        cs = segs[c]
        ta = cs[0][2] if cs else 0
        exps = [s[0] for s in cs]
        ea = exps[0] if exps else 0
        eb = exps[1] if len(exps) > 1 else ea
        for (e, tile_lo, ntk) in cs:
            src_lo = int(starts[e]) + tile_lo * MT
            src_hi = min(int(starts[e]) + counts[e], src_lo + ntk * MT)
            nrow = src_hi - src_lo
            xpt[:, slot * MT: slot * MT + nrow] = xT[:, src_lo:src_hi]
            place.append((slot, src_lo, nrow))
            slot += ntk
        placements.append(place)
        # [D, PAD_T] -> [128, nt, KC, MT] -> flat [128, nt*KC*MT]
        xqc = np.ascontiguousarray(
            xpt.reshape(KC, 128, nt, MT).transpose(1, 2, 0, 3)
        ).reshape(128, nt * KC * MT)
        in_maps.append({
            "xq": xqc,
            "wa1": w1q[ea], "wa2": w2q[ea].reshape(128, FC, D),
            "wa3": w3q[ea],
            "wb1": w1q[eb], "wb2": w2q[eb].reshape(128, FC, D),
            "wb3": w3q[eb],
            "meta": np.array([[ta]], dtype=np.int32),
        })

    trace = bool(int(os.environ.get("KERNEL_TRACE", "0")))
    try:
        res = run_bass_kernel_spmd(nc, in_maps, core_ids=list(range(NCORES)),
                                   trace=trace)
    except ModuleNotFoundError:
        res = run_bass_kernel_spmd(nc, in_maps, core_ids=list(range(NCORES)),
                                   trace=False)
    kernel.last_results = res

    out = np.empty((T, D), dtype=np.float32)
    for c in range(NCORES):
        o = res.results[c]["out"]
        for (slot, src_lo, nrow) in placements[c]:
            out[src_lo:src_lo + nrow] = \
                o[slot * MT: slot * MT + nrow].astype(np.float32)
    return out
